# revision 11
# baseline (speedup 1.0000x reference)
"""Causal self-attention (B=4, S=2048, D=1024, H=16) on 8 NeuronCores.

Sharding: core c handles batch b = c//2 and head-group g = c%2 (8 heads).
Each core computes qkv for its head group, causal attention for its 8 heads,
and a partial projection (its 512 rows of W_proj). Host sums the two partial
outputs per batch and adds b_proj.

Device layout (v4):
 - Heads are processed in PAIRS (2c, 2c+1). Head 2c lives at partitions 0:64,
   head 2c+1 at 64:128, so their scores matmuls (contraction = head dim 64)
   auto-derive PE row-tiles (0,0) and (64,0) and stream CONCURRENTLY on the
   two 64x128 halves of the PE array (2x scores throughput).
 - The causal mask is a 0/1 upper-triangle multiply on the 128-wide diagonal
   band of the exp'd probs, in-place on VectorE (no PE mode switch, no PSUM
   accumulation ordering constraint).
 - Scores PSUM spans are [128,512] (one bank), scp bufs=4, so the PE can run
   several spans ahead of the exp (ScalarE) drain without FIFO stalls; qkv/
   proj filler groups are emitted between scores and ctx to cover the drain.
 - ctx for the even head is interleaved per span (128x128 mode rounds); ctx
   for the odd head reads probs from a packed SBUF cache (41KB/partition)
   and is emitted as q-block groups at j=5/9/13/end - right when their probs
   slices complete and the even head's accumulator bank frees. PSUM stays at
   4 scores banks + 4 ctx accumulator banks.
 - denominators come from a ones-column appended to v (v_aug [k,65]); the
   ctx matmul then yields [ctx(64 rows); sums(1 row)] per q block.
 - normalization: reciprocal of the sums row, partition-broadcast on GpSimd,
   one fused multiply+cast on VectorE.
"""

import numpy as np
import ml_dtypes

import concourse.bacc as bacc
import concourse.tile as tile
from concourse import mybir
from concourse.bass_utils import run_bass_kernel_spmd

BF16 = mybir.dt.bfloat16
F32 = mybir.dt.float32
EXP = mybir.ActivationFunctionType.Exp

B = 4
S = 2048  # tokens per batch
D = 1024
HG = 8    # heads per core
HD = 64
GC = HG * HD  # 512 qkv columns per core per q/k/v
N_CORES = 8
SCALE = 0.125  # 1/sqrt(64)

# packed pT-cache offsets for the deferred (odd) head: per k-block j the
# unmasked q-range is [512*(j//4) + 128*(j%4), S) -> width 2048-512*(j//4)-128*(j%4)
_W = [2048 - 512 * (j // 4) - 128 * (j % 4) for j in range(16)]
_OFF = [sum(_W[:j]) for j in range(16)]
PTC = sum(_W)  # 20480


def _body(nc, xT, wq, wk, wv, wp, bqkv, tri, outT, tc, layout="fill", use_bias=True):
    _const_cm = tc.tile_pool(name="const", bufs=1)
    const = _const_cm.__enter__()
    qT_sb = const.tile([128, 4, S], BF16)
    kT_sb = const.tile([128, 4, S], BF16)
    ctxT_sb = const.tile([128, 4, S], BF16)
    vaug_sb = const.tile([128, 16, HG, 65], BF16)
    wp_sb = const.tile([128, 4, D], BF16)
    tri_sb = const.tile([128, 128], BF16)
    b_sb = const.tile([1, 3 * GC], BF16)
    ones1 = const.tile([1, 512], BF16)
    ptc_sb = const.tile([128, PTC], BF16)  # packed probs cache for odd heads

    nc.vector.memset(ones1[:], 1.0)
    nc.vector.memset(vaug_sb[:, :, :, 64:65], 1.0)
    nc.sync.dma_start(out=tri_sb[:], in_=tri.ap())
    nc.sync.dma_start(out=b_sb[:], in_=bqkv.ap())
    for ct in range(4):
        nc.sync.dma_start(out=wp_sb[:, ct, :], in_=wp.ap()[128 * ct:128 * (ct + 1), :])

    xT_sb = const.tile([128, 8, S], BF16)
    wq_sb = const.tile([128, 8, GC], BF16)
    wk_sb = const.tile([128, 8, GC], BF16)
    wv_sb = const.tile([128, 8, GC], BF16)
    # xT + wq stream first so the first qk matmuls can start ASAP
    for t in range(8):
        nc.sync.dma_start(out=xT_sb[:, t, :], in_=xT.ap()[128 * t:128 * (t + 1), :])
        nc.sync.dma_start(out=wq_sb[:, t, :], in_=wq.ap()[128 * t:128 * (t + 1), :])
    for t in range(8):
        nc.sync.dma_start(out=wk_sb[:, t, :], in_=wk.ap()[128 * t:128 * (t + 1), :])
    for t in range(8):
        nc.sync.dma_start(out=wv_sb[:, t, :], in_=wv.ap()[128 * t:128 * (t + 1), :])

    # PSUM: scp 4 x [128,512] (1 bank each) - scores spans, qkv, proj
    #       cxp 4 x [65,512]  (1 bank each) - ctx accumulators
    _scp_cm = tc.tile_pool(name="scp", bufs=4, space="PSUM")
    scp = _scp_cm.__enter__()
    _cxp_cm = tc.tile_pool(name="cxp", bufs=4, space="PSUM")
    cxp = _cxp_cm.__enter__()
    _prp_cm = tc.tile_pool(name="prp", bufs=3)
    prp = _prp_cm.__enter__()
    _nrm_cm = tc.tile_pool(name="nrm", bufs=4)
    nrm = _nrm_cm.__enter__()

    def qk_group(c, qk, tb):
        w_sb, dst, boff = ((wq_sb, qT_sb, 0), (wk_sb, kT_sb, GC))[qk]
        ps = scp.tile([128, 512], F32, tag="sc", name=f"qk_{c}_{boff}_{tb}")
        for t in range(8):
            nc.tensor.matmul(
                ps[:],
                lhsT=w_sb[:, t, 128 * c:128 * (c + 1)],
                rhs=xT_sb[:, t, 512 * tb:512 * (tb + 1)],
                start=(t == 0), stop=(not use_bias and t == 7))
        if use_bias:
            nc.tensor.matmul(
                ps[:],
                lhsT=b_sb[0:1, boff + 128 * c: boff + 128 * (c + 1)],
                rhs=ones1[0:1, :],
                start=False, stop=True)
        nc.vector.tensor_copy(dst[:, c, 512 * tb:512 * (tb + 1)], ps[:])

    def v_tile(j):
        # v in natural [token, v_col] layout, + bias, scattered into v_aug
        psv = scp.tile([128, 512], F32, tag="sc", name=f"pv_{j}")
        for t in range(8):
            nc.tensor.matmul(
                psv[:],
                lhsT=xT_sb[:, t, 128 * j:128 * (j + 1)],
                rhs=wv_sb[:, t, :],
                start=(t == 0), stop=(not use_bias and t == 7))
        if use_bias:
            nc.tensor.matmul(
                psv[:],
                lhsT=ones1[0:1, 0:128],
                rhs=b_sb[0:1, 2 * GC:3 * GC],
                start=False, stop=True)
        nc.vector.tensor_copy(
            vaug_sb[:, j, :, 0:64],
            psv[:].rearrange("p (h c) -> p h c", h=HG))

    def normalize(h, qb, ctx_ps):
        o = 64 * (h % 2)
        c = h // 2
        rec = nrm.tile([1, 512], F32, tag="rec", name=f"rec_{h}_{qb}")
        nc.vector.reciprocal(rec[:], ctx_ps[qb][64:65, :])
        bc = nrm.tile([64, 512], F32, tag="bc", name=f"bc_{h}_{qb}")
        nc.gpsimd.partition_broadcast(bc[:], rec[:])
        if o == 0:
            nc.vector.tensor_mul(
                ctxT_sb[0:64, c, 512 * qb:512 * (qb + 1)],
                ctx_ps[qb][0:64, :], bc[:])
        else:
            stg = nrm.tile([64, 512], BF16, tag="stg", name=f"stg_{h}_{qb}")
            nc.vector.tensor_mul(stg[:], ctx_ps[qb][0:64, :], bc[:])
            nc.sync.dma_start(
                out=ctxT_sb[64:128, c, 512 * qb:512 * (qb + 1)], in_=stg[:])

    _ob3_cm = tc.tile_pool(name="ob3", bufs=3)
    ob3 = _ob3_cm.__enter__()

    def proj_group(m, tb):
        ps = scp.tile([128, 512], F32, tag="sc", name=f"p3_{m}_{tb}")
        for ct in range(4):
            nc.tensor.matmul(
                ps[:],
                lhsT=wp_sb[:, ct, 128 * m:128 * (m + 1)],
                rhs=ctxT_sb[:, ct, 512 * tb:512 * (tb + 1)],
                start=(ct == 0), stop=(ct == 3))
        ob = ob3.tile([128, 512], F32, tag="o3", name=f"ob_{m}_{tb}")
        # ACT is loaded with exp; keep the psum drain off it where possible
        nc.scalar.copy(ob[:], ps[:])
        nc.sync.dma_start(
            out=outT.ap()[128 * m:128 * (m + 1), 512 * tb:512 * (tb + 1)],
            in_=ob[:])

    def pair_block(c, filler=None, defer_filler=None):
        """Heads h=2c (partitions 0:64, PE tile (0,0)) and h'=2c+1 (64:128,
        tile (64,0)). Scores stream concurrently on the two row-tiles; ctx of
        h interleaves per span, ctx of h' is deferred via the packed SBUF
        probs cache."""
        h0, h1 = 2 * c, 2 * c + 1
        ctx_ps = [cxp.tile([65, 512], F32, tag="cx", name=f"cx_{h0}_{qb}")
                  for qb in range(4)]

        def deferred_group(qb):
            # ctx + normalize for the odd head from the probs cache; emitted
            # as soon as its slices (j <= 4qb+3) are complete, reusing the
            # accumulator slot the even head's q-block qb just freed
            ps1 = {qb: cxp.tile([65, 512], F32, tag="cx", name=f"cx_{h1}_{qb}")}
            for j in range(4 * qb + 4):
                jq, jr = divmod(j, 4)
                jrel0 = 128 * jr
                lo = jrel0 if qb == jq else 0
                x0 = _OFF[j] + 512 * (qb - jq) - (0 if qb == jq else jrel0)
                nc.tensor.matmul(
                    ps1[qb][:, lo:512],
                    lhsT=vaug_sb[:, j, h1, :],
                    rhs=ptc_sb[:, x0:x0 + 512 - lo],
                    start=(j == 0), stop=(j == 4 * qb + 3))
            normalize(h1, qb, ps1)
            if defer_filler is not None:
                defer_filler(qb)
        for j in range(16):
            qbm, r = divmod(j, 4)
            rel0 = 128 * r
            nspans = 4 - qbm
            pT = prp.tile([128, S], BF16, tag="probs", name=f"pT_{h0}_{j}")
            # rounds of up to 2 span-pairs: 64x128-mode scores (+mask on the
            # diagonal span), exp per span, then 128x128-mode ctx for head h0
            for srnd in range(0, nspans, 2):
                spans = [s for s in (srnd, srnd + 1) if s < nspans]
                ps_t = {}
                for s in spans:
                    qb = qbm + s
                    lo = rel0 if s == 0 else 0
                    for o in (0, 64):
                        ps = scp.tile([128, 512], F32, tag="sc",
                                      name=f"sc_{h0 + o // 64}_{j}_{s}")
                        ps_t[(s, o)] = ps
                        nc.tensor.matmul(
                            ps[:, lo:512],
                            lhsT=kT_sb[o:o + 64, c, 128 * j:128 * (j + 1)],
                            rhs=qT_sb[o:o + 64, c,
                                      512 * qb + lo: 512 * (qb + 1)],
                            start=True, stop=True, skip_group_check=True)
                for s in spans:
                    qb = qbm + s
                    lo = rel0 if s == 0 else 0
                    nc.scalar.activation(
                        pT[:, 512 * s + lo:512 * (s + 1)],
                        ps_t[(s, 0)][:, lo:512], EXP, scale=SCALE)
                    x0 = _OFF[j] + 512 * s + lo - rel0
                    nc.scalar.activation(
                        ptc_sb[:, x0:x0 + 512 - lo],
                        ps_t[(s, 64)][:, lo:512], EXP, scale=SCALE)
                    if s == 0:
                        # causal mask: zero the upper triangle of the
                        # 128-wide diagonal band (probs cols [rel0,rel0+128)
                        # = k-block-relative q) in-place on VectorE
                        nc.vector.tensor_mul(
                            pT[:, rel0:rel0 + 128],
                            pT[:, rel0:rel0 + 128], tri_sb[:])
                        nc.vector.tensor_mul(
                            ptc_sb[:, _OFF[j]:_OFF[j] + 128],
                            ptc_sb[:, _OFF[j]:_OFF[j] + 128], tri_sb[:])
                # filler first: its matmuls cover the exp drain latency
                if srnd == 0 and filler is not None:
                    filler(j)
                # ctx: diag span last, so its mask-mul has time to drain
                for s in (spans[1:] + spans[:1] if srnd == 0 else spans):
                    qb = qbm + s
                    lo = rel0 if s == 0 else 0
                    nc.tensor.matmul(
                        ctx_ps[qb][:, lo:512],
                        lhsT=vaug_sb[:, j, h0, :],
                        rhs=pT[:, 512 * s + lo: 512 * (s + 1)],
                        start=(j == 0), stop=(j == 4 * qb + 3))
                    if s == 0 and r == 3:
                        normalize(h0, qb, ctx_ps)
            if j in (5, 9, 13):
                deferred_group((j - 5) // 4)
        deferred_group(3)

    qkg = [[(lambda c=c, qk=qk, tb=tb: qk_group(c, qk, tb))
            for qk in range(2) for tb in range(4)] for c in range(4)]

    def p0_filler(j):
        if j < 14:
            v_tile(j + 2)
        if j % 2 == 0:
            qkg[1][j // 2]()

    def mk_spread(groups):
        def f(j):
            if j % 2 == 0:
                groups[j // 2]()
        return f

    def p3_defer_filler(qb):
        for m in range(8):
            proj_group(m, qb)

    if layout == "fill":
        for g in qkg[0]:
            g()
        v_tile(0)
        v_tile(1)
        pair_block(0, filler=p0_filler)
        pair_block(1, filler=mk_spread(qkg[2]))
        pair_block(2, filler=mk_spread(qkg[3]))
        pair_block(3, defer_filler=p3_defer_filler)
    else:
        raise ValueError(layout)

    _ob3_cm.__exit__(None, None, None)
    _nrm_cm.__exit__(None, None, None)
    _prp_cm.__exit__(None, None, None)
    _cxp_cm.__exit__(None, None, None)
    _scp_cm.__exit__(None, None, None)
    _const_cm.__exit__(None, None, None)


_CACHED = {}


def _build(reps=1, layout="fill", use_bias=True):
    key = (reps, layout, use_bias)
    if key in _CACHED:
        return _CACHED[key]
    nc = bacc.Bacc()
    xT = nc.dram_tensor("xT", [D, S], BF16, kind="ExternalInput")
    wq = nc.dram_tensor("wq", [D, GC], BF16, kind="ExternalInput")
    wk = nc.dram_tensor("wk", [D, GC], BF16, kind="ExternalInput")
    wv = nc.dram_tensor("wv", [D, GC], BF16, kind="ExternalInput")
    wp = nc.dram_tensor("wp", [GC, D], BF16, kind="ExternalInput")
    bqkv = nc.dram_tensor("bqkv", [1, 3 * GC], BF16, kind="ExternalInput")
    tri = nc.dram_tensor("tri", [128, 128], BF16, kind="ExternalInput")
    outT = nc.dram_tensor("outT", [D, S], F32, kind="ExternalOutput")
    with tile.TileContext(nc) as tc:
        for _ in range(reps):
            _body(nc, xT, wq, wk, wv, wp, bqkv, tri, outT, tc, layout=layout, use_bias=use_bias)
    nc.compile()
    _CACHED[key] = nc
    return nc


def make_in_maps(x, W_attn, b_attn, W_proj):
    bf = ml_dtypes.bfloat16
    # 0/1 keep-mask for the 128-wide diagonal band: col x (= q rel to the
    # k-block base) is kept for k-partition p iff x >= p
    tri_np = (np.arange(128)[None, :] >= np.arange(128)[:, None]).astype(bf)
    in_maps = []
    for core in range(N_CORES):
        b, g = divmod(core, 2)
        cols = slice(GC * g, GC * (g + 1))
        in_maps.append({
            "xT": np.ascontiguousarray(x[b].T).astype(bf),
            "wq": np.ascontiguousarray(W_attn[:, cols]).astype(bf),
            "wk": np.ascontiguousarray(W_attn[:, D:][:, cols]).astype(bf),
            "wv": np.ascontiguousarray(W_attn[:, 2 * D:][:, cols]).astype(bf),
            "wp": np.ascontiguousarray(W_proj[cols, :]).astype(bf),
            "bqkv": np.concatenate(
                [b_attn[cols], b_attn[D:][cols], b_attn[2 * D:][cols]]
            ).reshape(1, 3 * GC).astype(bf),
            "tri": tri_np,
        })
    return in_maps


def kernel(x, W_attn, b_attn, W_proj, b_proj, _run_kwargs=None):
    x = np.asarray(x)
    W_attn = np.asarray(W_attn)
    b_attn = np.asarray(b_attn)
    W_proj = np.asarray(W_proj)
    b_proj = np.asarray(b_proj)

    use_bias = bool(np.any(b_attn))
    nc = _build(use_bias=use_bias)
    in_maps = make_in_maps(x, W_attn, b_attn, W_proj)

    res = run_bass_kernel_spmd(
        nc, in_maps, core_ids=list(range(N_CORES)), **(_run_kwargs or {}))

    out = np.empty((B, S, D), np.float32)
    for b in range(B):
        acc = res.results[2 * b]["outT"] + res.results[2 * b + 1]["outT"]
        out[b] = acc.T + b_proj[None, :].astype(np.float32)
    if _run_kwargs:
        kernel.last_results = res
    return out


# revision 12
# speedup vs baseline: 3.2119x; 3.2119x over previous
"""Causal self-attention (B=4, S=2048, D=1024, H=16) on 8 NeuronCores.

Sharding: core c handles batch b = c//2 and head-group g = c%2 (8 heads).
Each core computes qkv for its head group, causal attention for its 8 heads,
and a partial projection (its 512 rows of W_proj). Host sums the two partial
outputs per batch and adds b_proj.

Device layout (v4):
 - Heads are processed in PAIRS (2c, 2c+1). Head 2c lives at partitions 0:64,
   head 2c+1 at 64:128, so their scores matmuls (contraction = head dim 64)
   auto-derive PE row-tiles (0,0) and (64,0) and stream CONCURRENTLY on the
   two 64x128 halves of the PE array (2x scores throughput).
 - The causal mask is a 0/1 upper-triangle multiply on the 128-wide diagonal
   band of the exp'd probs, in-place on VectorE (no PE mode switch, no PSUM
   accumulation ordering constraint).
 - Scores PSUM spans are [128,512] (one bank), scp bufs=4, so the PE can run
   several spans ahead of the exp (ScalarE) drain without FIFO stalls; qkv/
   proj filler groups are emitted between scores and ctx to cover the drain.
 - ctx for the even head is interleaved per span (128x128 mode rounds); ctx
   for the odd head reads probs from a packed SBUF cache (41KB/partition)
   and is emitted as q-block groups at j=5/9/13/end - right when their probs
   slices complete and the even head's accumulator bank frees. PSUM stays at
   4 scores banks + 4 ctx accumulator banks.
 - denominators come from a ones-column appended to v (v_aug [k,65]); the
   ctx matmul then yields [ctx(64 rows); sums(1 row)] per q block.
 - normalization: reciprocal of the sums row, partition-broadcast on GpSimd,
   one fused multiply+cast on VectorE.
"""

import numpy as np
import ml_dtypes

import concourse.bacc as bacc
import concourse.tile as tile
from concourse import mybir
from concourse.bass_utils import run_bass_kernel_spmd

BF16 = mybir.dt.bfloat16
F32 = mybir.dt.float32
EXP = mybir.ActivationFunctionType.Exp

B = 4
S = 2048  # tokens per batch
D = 1024
HG = 8    # heads per core
HD = 64
GC = HG * HD  # 512 qkv columns per core per q/k/v
N_CORES = 8
SCALE = 0.125  # 1/sqrt(64)

# packed pT-cache offsets for the deferred (odd) head: per k-block j the
# unmasked q-range is [512*(j//4) + 128*(j%4), S) -> width 2048-512*(j//4)-128*(j%4)
_W = [2048 - 512 * (j // 4) - 128 * (j % 4) for j in range(16)]
_OFF = [sum(_W[:j]) for j in range(16)]
PTC = sum(_W)  # 20480


def _body(nc, xT, wq, wk, wv, wp, bqkv, tri, outT, tc, layout="fill", use_bias=True):
    _const_cm = tc.tile_pool(name="const", bufs=1)
    const = _const_cm.__enter__()
    qT_sb = const.tile([128, 4, S], BF16)
    kT_sb = const.tile([128, 4, S], BF16)
    ctxT_sb = const.tile([128, 4, S], BF16)
    vaug_sb = const.tile([128, 16, HG, 65], BF16)
    wp_sb = const.tile([128, 4, D], BF16)
    tri_sb = const.tile([128, 128], BF16)
    b_sb = const.tile([1, 3 * GC], BF16)
    ones1 = const.tile([1, 512], BF16)
    ptc_sb = const.tile([128, PTC], BF16)  # packed probs cache for odd heads

    nc.vector.memset(ones1[:], 1.0)
    nc.vector.memset(vaug_sb[:, :, :, 64:65], 1.0)
    nc.sync.dma_start(out=tri_sb[:], in_=tri.ap())
    nc.sync.dma_start(out=b_sb[:], in_=bqkv.ap())
    for ct in range(4):
        nc.sync.dma_start(out=wp_sb[:, ct, :], in_=wp.ap()[128 * ct:128 * (ct + 1), :])

    xT_sb = const.tile([128, 8, S], BF16)
    wq_sb = const.tile([128, 8, GC], BF16)
    wk_sb = const.tile([128, 8, GC], BF16)
    wv_sb = const.tile([128, 8, GC], BF16)
    # xT + wq stream first so the first qk matmuls can start ASAP
    for t in range(8):
        nc.sync.dma_start(out=xT_sb[:, t, :], in_=xT.ap()[128 * t:128 * (t + 1), :])
        nc.sync.dma_start(out=wq_sb[:, t, :], in_=wq.ap()[128 * t:128 * (t + 1), :])
    for t in range(8):
        nc.sync.dma_start(out=wk_sb[:, t, :], in_=wk.ap()[128 * t:128 * (t + 1), :])
    for t in range(8):
        nc.sync.dma_start(out=wv_sb[:, t, :], in_=wv.ap()[128 * t:128 * (t + 1), :])

    # PSUM: scp 4 x [128,512] (1 bank each) - scores spans, qkv, proj
    #       cxp 4 x [65,512]  (1 bank each) - ctx accumulators
    _scp_cm = tc.tile_pool(name="scp", bufs=4, space="PSUM")
    scp = _scp_cm.__enter__()
    _cxp_cm = tc.tile_pool(name="cxp", bufs=4, space="PSUM")
    cxp = _cxp_cm.__enter__()
    _prp_cm = tc.tile_pool(name="prp", bufs=3)
    prp = _prp_cm.__enter__()
    _nrm_cm = tc.tile_pool(name="nrm", bufs=4)
    nrm = _nrm_cm.__enter__()

    def qk_group(c, qk, tb):
        w_sb, dst, boff = ((wq_sb, qT_sb, 0), (wk_sb, kT_sb, GC))[qk]
        ps = scp.tile([128, 512], F32, tag="sc", name=f"qk_{c}_{boff}_{tb}")
        for t in range(8):
            nc.tensor.matmul(
                ps[:],
                lhsT=w_sb[:, t, 128 * c:128 * (c + 1)],
                rhs=xT_sb[:, t, 512 * tb:512 * (tb + 1)],
                start=(t == 0), stop=(not use_bias and t == 7))
        if use_bias:
            nc.tensor.matmul(
                ps[:],
                lhsT=b_sb[0:1, boff + 128 * c: boff + 128 * (c + 1)],
                rhs=ones1[0:1, :],
                start=False, stop=True)
        nc.vector.tensor_copy(dst[:, c, 512 * tb:512 * (tb + 1)], ps[:])

    def v_tile(j):
        # v in natural [token, v_col] layout, + bias, scattered into v_aug
        psv = scp.tile([128, 512], F32, tag="sc", name=f"pv_{j}")
        for t in range(8):
            nc.tensor.matmul(
                psv[:],
                lhsT=xT_sb[:, t, 128 * j:128 * (j + 1)],
                rhs=wv_sb[:, t, :],
                start=(t == 0), stop=(not use_bias and t == 7))
        if use_bias:
            nc.tensor.matmul(
                psv[:],
                lhsT=ones1[0:1, 0:128],
                rhs=b_sb[0:1, 2 * GC:3 * GC],
                start=False, stop=True)
        nc.vector.tensor_copy(
            vaug_sb[:, j, :, 0:64],
            psv[:].rearrange("p (h c) -> p h c", h=HG))

    def normalize(h, qb, ctx_ps):
        o = 64 * (h % 2)
        c = h // 2
        rec = nrm.tile([1, 512], F32, tag="rec", name=f"rec_{h}_{qb}")
        nc.vector.reciprocal(rec[:], ctx_ps[qb][64:65, :])
        bc = nrm.tile([64, 512], F32, tag="bc", name=f"bc_{h}_{qb}")
        nc.gpsimd.partition_broadcast(bc[:], rec[:])
        if o == 0:
            nc.vector.tensor_mul(
                ctxT_sb[0:64, c, 512 * qb:512 * (qb + 1)],
                ctx_ps[qb][0:64, :], bc[:])
        else:
            stg = nrm.tile([64, 512], BF16, tag="stg", name=f"stg_{h}_{qb}")
            nc.vector.tensor_mul(stg[:], ctx_ps[qb][0:64, :], bc[:])
            nc.sync.dma_start(
                out=ctxT_sb[64:128, c, 512 * qb:512 * (qb + 1)], in_=stg[:])

    _ob3_cm = tc.tile_pool(name="ob3", bufs=3)
    ob3 = _ob3_cm.__enter__()

    def proj_group(m, tb):
        ps = scp.tile([128, 512], F32, tag="sc", name=f"p3_{m}_{tb}")
        for ct in range(4):
            nc.tensor.matmul(
                ps[:],
                lhsT=wp_sb[:, ct, 128 * m:128 * (m + 1)],
                rhs=ctxT_sb[:, ct, 512 * tb:512 * (tb + 1)],
                start=(ct == 0), stop=(ct == 3))
        ob = ob3.tile([128, 512], F32, tag="o3", name=f"ob_{m}_{tb}")
        # ACT is loaded with exp; keep the psum drain off it where possible
        nc.scalar.copy(ob[:], ps[:])
        nc.sync.dma_start(
            out=outT.ap()[128 * m:128 * (m + 1), 512 * tb:512 * (tb + 1)],
            in_=ob[:])

    def pair_block(c, filler=None, defer_filler=None):
        """Heads h=2c (partitions 0:64, PE tile (0,0)) and h'=2c+1 (64:128,
        tile (64,0)). Scores stream concurrently on the two row-tiles; ctx of
        h interleaves per span, ctx of h' is deferred via the packed SBUF
        probs cache."""
        h0, h1 = 2 * c, 2 * c + 1
        ctx_ps = [cxp.tile([65, 512], F32, tag="cx", name=f"cx_{h0}_{qb}")
                  for qb in range(4)]

        def deferred_group(qb):
            # ctx + normalize for the odd head from the probs cache; emitted
            # as soon as its slices (j <= 4qb+3) are complete, reusing the
            # accumulator slot the even head's q-block qb just freed
            ps1 = {qb: cxp.tile([65, 512], F32, tag="cx", name=f"cx_{h1}_{qb}")}
            for j in range(4 * qb + 4):
                jq, jr = divmod(j, 4)
                jrel0 = 128 * jr
                lo = jrel0 if qb == jq else 0
                x0 = _OFF[j] + 512 * (qb - jq) - (0 if qb == jq else jrel0)
                nc.tensor.matmul(
                    ps1[qb][:, lo:512],
                    lhsT=vaug_sb[:, j, h1, :],
                    rhs=ptc_sb[:, x0:x0 + 512 - lo],
                    start=(j == 0), stop=(j == 4 * qb + 3))
            normalize(h1, qb, ps1)
            if defer_filler is not None:
                defer_filler(qb)
        for j in range(16):
            qbm, r = divmod(j, 4)
            rel0 = 128 * r
            nspans = 4 - qbm
            pT = prp.tile([128, S], BF16, tag="probs", name=f"pT_{h0}_{j}")
            # rounds of up to 2 span-pairs: 64x128-mode scores (+mask on the
            # diagonal span), exp per span, then 128x128-mode ctx for head h0
            for srnd in range(0, nspans, 2):
                spans = [s for s in (srnd, srnd + 1) if s < nspans]
                ps_t = {}
                for s in spans:
                    qb = qbm + s
                    lo = rel0 if s == 0 else 0
                    for o in (0, 64):
                        ps = scp.tile([128, 512], F32, tag="sc",
                                      name=f"sc_{h0 + o // 64}_{j}_{s}")
                        ps_t[(s, o)] = ps
                        nc.tensor.matmul(
                            ps[:, lo:512],
                            lhsT=kT_sb[o:o + 64, c, 128 * j:128 * (j + 1)],
                            rhs=qT_sb[o:o + 64, c,
                                      512 * qb + lo: 512 * (qb + 1)],
                            start=True, stop=True, skip_group_check=True)
                for s in spans:
                    qb = qbm + s
                    lo = rel0 if s == 0 else 0
                    nc.scalar.activation(
                        pT[:, 512 * s + lo:512 * (s + 1)],
                        ps_t[(s, 0)][:, lo:512], EXP, scale=SCALE)
                    x0 = _OFF[j] + 512 * s + lo - rel0
                    nc.scalar.activation(
                        ptc_sb[:, x0:x0 + 512 - lo],
                        ps_t[(s, 64)][:, lo:512], EXP, scale=SCALE)
                    if s == 0:
                        # causal mask: zero the upper triangle of the
                        # 128-wide diagonal band (probs cols [rel0,rel0+128)
                        # = k-block-relative q) in-place on VectorE
                        nc.vector.tensor_mul(
                            pT[:, rel0:rel0 + 128],
                            pT[:, rel0:rel0 + 128], tri_sb[:])
                        nc.vector.tensor_mul(
                            ptc_sb[:, _OFF[j]:_OFF[j] + 128],
                            ptc_sb[:, _OFF[j]:_OFF[j] + 128], tri_sb[:])
                # filler first: its matmuls cover the exp drain latency
                if srnd == 0 and filler is not None:
                    filler(j)
                # ctx: diag span last, so its mask-mul has time to drain
                for s in (spans[1:] + spans[:1] if srnd == 0 else spans):
                    qb = qbm + s
                    lo = rel0 if s == 0 else 0
                    nc.tensor.matmul(
                        ctx_ps[qb][:, lo:512],
                        lhsT=vaug_sb[:, j, h0, :],
                        rhs=pT[:, 512 * s + lo: 512 * (s + 1)],
                        start=(j == 0), stop=(j == 4 * qb + 3))
                    if s == 0 and r == 3:
                        normalize(h0, qb, ctx_ps)
            if j in (5, 9, 13):
                deferred_group((j - 5) // 4)
        deferred_group(3)

    # tb-major, qk-interleaved: pair c's j=0 scores only need (q,tb0)+(k,tb0)
    qkg = [[(lambda c=c, qk=qk, tb=tb: qk_group(c, qk, tb))
            for tb in range(4) for qk in range(2)] for c in range(4)]

    def p0_filler(j):
        if j < 14:
            v_tile(j + 2)
        if j % 2 == 0:
            qkg[1][j // 2]()

    def mk_spread(groups):
        def f(j):
            if j % 2 == 0:
                groups[j // 2]()
        return f

    def p3_defer_filler(qb):
        for m in range(8):
            proj_group(m, qb)

    if layout == "fill":
        for g in qkg[0]:
            g()
        v_tile(0)
        v_tile(1)
        pair_block(0, filler=p0_filler)
        pair_block(1, filler=mk_spread(qkg[2]))
        pair_block(2, filler=mk_spread(qkg[3]))
        pair_block(3, defer_filler=p3_defer_filler)
    else:
        raise ValueError(layout)

    _ob3_cm.__exit__(None, None, None)
    _nrm_cm.__exit__(None, None, None)
    _prp_cm.__exit__(None, None, None)
    _cxp_cm.__exit__(None, None, None)
    _scp_cm.__exit__(None, None, None)
    _const_cm.__exit__(None, None, None)


_CACHED = {}


def _build(reps=1, layout="fill", use_bias=True):
    key = (reps, layout, use_bias)
    if key in _CACHED:
        return _CACHED[key]
    nc = bacc.Bacc()
    xT = nc.dram_tensor("xT", [D, S], BF16, kind="ExternalInput")
    wq = nc.dram_tensor("wq", [D, GC], BF16, kind="ExternalInput")
    wk = nc.dram_tensor("wk", [D, GC], BF16, kind="ExternalInput")
    wv = nc.dram_tensor("wv", [D, GC], BF16, kind="ExternalInput")
    wp = nc.dram_tensor("wp", [GC, D], BF16, kind="ExternalInput")
    bqkv = nc.dram_tensor("bqkv", [1, 3 * GC], BF16, kind="ExternalInput")
    tri = nc.dram_tensor("tri", [128, 128], BF16, kind="ExternalInput")
    outT = nc.dram_tensor("outT", [D, S], F32, kind="ExternalOutput")
    with tile.TileContext(nc) as tc:
        for _ in range(reps):
            _body(nc, xT, wq, wk, wv, wp, bqkv, tri, outT, tc, layout=layout, use_bias=use_bias)
    nc.compile()
    _CACHED[key] = nc
    return nc


def make_in_maps(x, W_attn, b_attn, W_proj):
    bf = ml_dtypes.bfloat16
    # 0/1 keep-mask for the 128-wide diagonal band: col x (= q rel to the
    # k-block base) is kept for k-partition p iff x >= p
    tri_np = (np.arange(128)[None, :] >= np.arange(128)[:, None]).astype(bf)
    in_maps = []
    for core in range(N_CORES):
        b, g = divmod(core, 2)
        cols = slice(GC * g, GC * (g + 1))
        in_maps.append({
            "xT": np.ascontiguousarray(x[b].T).astype(bf),
            "wq": np.ascontiguousarray(W_attn[:, cols]).astype(bf),
            "wk": np.ascontiguousarray(W_attn[:, D:][:, cols]).astype(bf),
            "wv": np.ascontiguousarray(W_attn[:, 2 * D:][:, cols]).astype(bf),
            "wp": np.ascontiguousarray(W_proj[cols, :]).astype(bf),
            "bqkv": np.concatenate(
                [b_attn[cols], b_attn[D:][cols], b_attn[2 * D:][cols]]
            ).reshape(1, 3 * GC).astype(bf),
            "tri": tri_np,
        })
    return in_maps


def kernel(x, W_attn, b_attn, W_proj, b_proj, _run_kwargs=None):
    x = np.asarray(x)
    W_attn = np.asarray(W_attn)
    b_attn = np.asarray(b_attn)
    W_proj = np.asarray(W_proj)
    b_proj = np.asarray(b_proj)

    use_bias = bool(np.any(b_attn))
    nc = _build(use_bias=use_bias)
    in_maps = make_in_maps(x, W_attn, b_attn, W_proj)

    res = run_bass_kernel_spmd(
        nc, in_maps, core_ids=list(range(N_CORES)), **(_run_kwargs or {}))

    out = np.empty((B, S, D), np.float32)
    for b in range(B):
        acc = res.results[2 * b]["outT"] + res.results[2 * b + 1]["outT"]
        out[b] = acc.T + b_proj[None, :].astype(np.float32)
    if _run_kwargs:
        kernel.last_results = res
    return out
